# revision 36
# baseline (speedup 1.0000x reference)
"""Trainium2 Bass kernel for BasicCNN_LSTM (3x conv3x3+relu -> BN -> GAP -> LSTM -> BN -> dense).

Sharding: data-parallel over batch across 8 NeuronCores (4 batches/core).

Per-core plan (128 frames = 4 batches x 32 timesteps, processed as 64 frame-pairs):
  - conv1 (C=1 -> 48): host-built im2col [18, N] block-diagonal matmul (two 14-row
    chunks of one frame per column stream).
  - conv2/conv3 (48 -> 48): 9 tap-accumulated bf16 matmuls (K=48), 4 PE quadrant
    streams per tap; activations stored split: pixel-row-half 0 at SBUF partitions
    0:48, half 1 at partitions 64:112, each padded [16, 30] bf16.
  - PSUM banks are frame-major: bank PA = frame 0 (rows 0..13 at psum partitions
    0:64 from half0, rows 14..27 at 64:128 from half1), bank PB = frame 1. This
    makes the psum->act-buffer store a single [112]-partition op per frame plus
    two 28-col seam slivers, and GAP a single accum ACT per frame.
  - conv bias via activation bias operand; BN1 folded into LSTM input weights;
    GAP via activation accum_out written directly into Z columns; /784 in weights.
  - LSTM: one K=113 matmul per step (rhs Z = [112 pooled sums; ones row 112]);
    sigmoid/tanh on ACT, elementwise on DVE. BN2 + output dense folded into a
    final K=9 matmul over the stored h sequence.
"""

import sys

sys.path.insert(0, "/opt/trn_rl_repo")

import numpy as np
import ml_dtypes

_BF16 = ml_dtypes.bfloat16
_NCORES = 8
_B, _T, _HW, _F, _U = 32, 32, 28, 48, 8
_EPS = 1e-3
_BPC = _B // _NCORES          # batches per core (4)
_PAIRS = _BPC * _T // 2       # frame pairs per core (64)

_F32 = np.float32


# ---------------------------------------------------------------------------
# Device program
# ---------------------------------------------------------------------------

def _build_program():
    import concourse.bass as bass  # noqa: F401
    import concourse.tile as tile
    from concourse.tile import add_dep_helper
    from concourse import bacc, mybir

    f32 = mybir.dt.float32
    bf16 = mybir.dt.bfloat16
    AF = mybir.ActivationFunctionType
    ALU = mybir.AluOpType

    nc = bacc.Bacc("TRN2", target_bir_lowering=False, debug=False, num_devices=_NCORES)

    # DRAM I/O
    x1_d = nc.dram_tensor("x1", (_PAIRS, 2, 18, 392), bf16, kind="ExternalInput")
    w1_d = nc.dram_tensor("w1t", (128, 128), bf16, kind="ExternalInput")
    w2_d = nc.dram_tensor("w2t", (128, 9, 64), bf16, kind="ExternalInput")
    w3_d = nc.dram_tensor("w3t", (128, 9, 64), bf16, kind="ExternalInput")
    b1_d = nc.dram_tensor("b1t", (128, 1), f32, kind="ExternalInput")
    b2_d = nc.dram_tensor("b2t", (128, 1), f32, kind="ExternalInput")
    b3_d = nc.dram_tensor("b3t", (128, 1), f32, kind="ExternalInput")
    wf_d = nc.dram_tensor("wft", (128, 128), f32, kind="ExternalInput")
    wh_d = nc.dram_tensor("wht", (128, 128), f32, kind="ExternalInput")
    bo_d = nc.dram_tensor("bot", (1, 1), f32, kind="ExternalInput")
    wo_d = nc.dram_tensor("woutt", (128, 1), f32, kind="ExternalInput")
    out_d = nc.dram_tensor("out", (1, 128), f32, kind="ExternalOutput")

    from contextlib import ExitStack
    with tile.TileContext(nc) as tc, ExitStack() as ctx:
        # ---- persistent tiles (one const pool, unique tag per tile) ----
        cp = ctx.enter_context(tc.tile_pool(name="const", bufs=1))
        W1T = cp.tile([128, 128], bf16, name="W1T", tag="W1T")
        W2T = cp.tile([128, 9, 64], bf16, name="W2T", tag="W2T")
        W3T = cp.tile([128, 9, 64], bf16, name="W3T", tag="W3T")
        B1T = cp.tile([128, 1], f32, name="B1T", tag="B1T")
        B2T = cp.tile([128, 1], f32, name="B2T", tag="B2T")
        B3T = cp.tile([128, 1], f32, name="B3T", tag="B3T")
        WFT = cp.tile([128, 128], f32, name="WFT", tag="WFT")
        WHT = cp.tile([128, 128], f32, name="WHT", tag="WHT")
        BOT = cp.tile([1, 1], f32, name="BOT", tag="BOT")
        WOT = cp.tile([128, 1], f32, name="WOT", tag="WOT")
        HST = cp.tile([128, 128], f32, name="HST", tag="HST")   # rows 0:8 h, col = 4t+b
        ZT = [cp.tile([128, 4], f32, name=f"ZT{i}", tag=f"ZT{i}") for i in range(3)]
        CT = [cp.tile([8, 4], f32, name=f"CT{i}", tag=f"CT{i}") for i in range(2)]
        HT = [cp.tile([8, 4], f32, name=f"HT{i}", tag=f"HT{i}") for i in range(2)]
        # persistent split act buffers (pads zeroed once)
        A1B = [cp.tile([128, 2, 16, 30], bf16, name=f"A1B{i}", tag=f"A1B{i}")
               for i in range(3)]
        A2B = [cp.tile([128, 2, 16, 30], bf16, name=f"A2B{i}", tag=f"A2B{i}")
               for i in range(3)]

        nc.sync.dma_start(W1T[:, :], w1_d.ap()[:, :])
        nc.gpsimd.dma_start(W2T[:, :, :], w2_d.ap()[:, :, :])
        nc.scalar.dma_start(W3T[:, :, :], w3_d.ap()[:, :, :])
        nc.sync.dma_start(B1T[:, :], b1_d.ap()[:, :])
        nc.gpsimd.dma_start(B2T[:, :], b2_d.ap()[:, :])
        nc.scalar.dma_start(B3T[:, :], b3_d.ap()[:, :])
        nc.gpsimd.dma_start(WFT[:, :], wf_d.ap()[:, :])
        nc.gpsimd.dma_start(WHT[:, :], wh_d.ap()[:, :])
        nc.scalar.dma_start(WOT[:, :], wo_d.ap()[:, :])
        nc.scalar.dma_start(BOT[:, :], bo_d.ap()[:, :])

        # Z rows 96:128 := 1.0 once (row 112 is the LSTM bias-ones row; rows
        # 0:112 are fully overwritten by pool accum_out before every read;
        # base partition must be a multiple of 32)
        for z in ZT:
            nc.vector.memset(z[96:128, :], 1.0)
        nc.vector.memset(CT[0][:, :], 0.0)
        nc.vector.memset(HT[0][:, :], 0.0)

        # one-time pad zeroing of the persistent act buffers:
        # half0 (p0:48): pad row 0; half1 (p64:112): pad row 15; cols 0, 29 both.
        for A in A1B + A2B:
            nc.gpsimd.memset(A[0:48, :, 0:1, :], 0.0)
            nc.gpsimd.memset(A[64:112, :, 15:16, :], 0.0)
            nc.gpsimd.memset(A[0:128, :, :, 0:1], 0.0)
            nc.gpsimd.memset(A[0:128, :, :, 29:30], 0.0)

        # ---- pools ----
        x1_pool = ctx.enter_context(tc.tile_pool(name="x1p", bufs=6))
        ps_pool = ctx.enter_context(tc.tile_pool(name="psp", bufs=6, space="PSUM"))
        g_pool = ctx.enter_context(tc.tile_pool(name="gp", bufs=2, space="PSUM"))
        ls_pool = ctx.enter_context(tc.tile_pool(name="lsp", bufs=4))

        # per-pair state carried between loop stages
        P1 = [None] * _PAIRS   # (PA, PB, mA, mB): bank = frame in pair
        P2 = [None] * _PAIRS
        P3 = [None] * _PAIRS

        def conv1(p):
            # block-diagonal K=18/M=112 packing: each streamed column carries
            # the 9 im2col taps of one frame; chunk rows 0..13 at lhsT rows
            # 0:9 -> psum 0:48, chunk rows 14..27 at rows 9:18 -> psum 64:112.
            # Bank PA = frame 0 of the pair, PB = frame 1.
            X = x1_pool.tile([128, 392], bf16, name="X")
            nc.sync.dma_start(X[0:18, :], x1_d.ap()[p, 0])
            nc.sync.dma_start(X[64:82, :], x1_d.ap()[p, 1])
            PAf = ps_pool.tile([128, 512], f32, name="PAf", tag="cps")
            PBf = ps_pool.tile([128, 512], f32, name="PBf", tag="cps")
            PA, PB = PAf[:, 0:392], PBf[:, 0:392]
            mA = nc.tensor.matmul(PA[0:112, :], lhsT=W1T[0:18, 0:112], rhs=X[0:18, :],
                                  skip_group_check=True)
            mB = nc.tensor.matmul(PB[0:112, :], lhsT=W1T[64:82, 0:112], rhs=X[64:82, :],
                                  skip_group_check=True)
            P1[p] = (PA, PB, mA, mB)

        def conv23(p, W, A, Pout):
            # bank = frame g. PA (frame 0): block 0:64 <- half0 (out rows
            # 0..13, lo), block 64:128 <- half1 (rows 14..27, hi). PB (frame
            # 1) is SWAPPED: block 0:64 <- half1 (rows 14..27, hi), block
            # 64:128 <- half0 (rows 0..13, lo) -- so the four matmuls of a
            # tap land on four distinct PE tiles (no same-tile back-to-back
            # streams, LDWEIGHTS stays in the background buffer).
            PAf = ps_pool.tile([128, 512], f32, name="PA23f", tag="cps")
            PBf = ps_pool.tile([128, 512], f32, name="PB23f", tag="cps")
            PA, PB = PAf[:, 0:392], PBf[:, 0:392]
            # dy=1 taps first: their windows (buf rows 1..14) only touch the
            # big relu_store writes, not the seam rows 0/15 -- so this pair's
            # first matmuls don't wait on the DVE seam chain.
            for ti, tap in enumerate((3, 4, 5, 0, 1, 2, 6, 7, 8)):
                dy, dx = tap // 3, tap % 3
                st = ti == 0
                sp = ti == 8
                lo = W[0:48, tap, :]
                hi = W[64:112, tap, :]
                # slot 1: tiles (0,0) + (64,64); slot 2: tiles (0,64) + (64,0)
                nc.tensor.matmul(PA[0:64, :], lhsT=lo,
                                 rhs=A[0:48, 0:1, dy:dy + 14, dx:dx + 28],
                                 start=st, stop=sp, skip_group_check=True)
                mA = nc.tensor.matmul(PA[64:128, :], lhsT=hi,
                                 rhs=A[64:112, 0:1, dy:dy + 14, dx:dx + 28],
                                 start=st, stop=sp, skip_group_check=True)
                mB = nc.tensor.matmul(PB[64:128, :], lhsT=lo,
                                 rhs=A[0:48, 1:2, dy:dy + 14, dx:dx + 28],
                                 start=st, stop=sp, skip_group_check=True)
                nc.tensor.matmul(PB[0:64, :], lhsT=hi,
                                 rhs=A[64:112, 1:2, dy:dy + 14, dx:dx + 28],
                                 start=st, stop=sp, skip_group_check=True)
            Pout[p] = (PA, PB, mA, mB)

        def relu_store(p, Psrc, B, Abufs, Adst_out):
            # psum (frame-banked) -> padded split act buffer, relu + bias.
            PA, PB, mA, mB = Psrc[p]
            A = Abufs[p % 3]
            # big write: whole frame, both halves in one [112]-partition op.
            # buf0 rows 1..14 <- out rows 0..13 (block 0:64); buf1 rows 1..14
            # <- out rows 14..27 (block 64:128); junk lands on partitions
            # 48:64 (unused gap).
            pa4 = PA.rearrange("p (a r c) -> p a r c", a=1, r=14, c=28)
            pb4 = PB.rearrange("p (a r c) -> p a r c", a=1, r=14, c=28)
            # frame 0 (PA, straight): one [112]-partition write covers both
            # halves (buf rows 1..14 in each); junk lands on partitions 48:64.
            i1 = nc.scalar.activation(A[0:112, 0:1, 1:15, 1:29], pa4[0:112, :, :, :],
                                      AF.Relu, bias=B[0:112, :])
            # frame 1 (PB, swapped blocks): two crossed half-writes. Both are
            # partition-base-shifted (in base != out base), which only the
            # DVE handles; the ACT takes the base-aligned seam slivers.
            i2a = nc.vector.tensor_scalar(A[0:48, 1:2, 1:15, 1:29],
                                          pb4[64:112, :, :, :],
                                          B[64:112, :], 0.0, ALU.add, ALU.max)
            i2b = nc.vector.tensor_scalar(A[64:112, 1:2, 1:15, 1:29],
                                          pb4[0:48, :, :, :],
                                          B[0:48, :], 0.0, ALU.add, ALU.max)
            # seam slivers: buf0 row 15 = out row 14 (first 28 cols of the
            # rows-14..27 block); buf1 row 0 = out row 13 (last 28 cols of
            # the rows-0..13 block).
            i3 = nc.vector.tensor_scalar(A[0:48, 0:1, 15:16, 1:29],
                                         pa4[64:112, :, 0:1, :],
                                         B[64:112, :], 0.0, ALU.add, ALU.max)
            i4 = nc.vector.tensor_scalar(A[64:112, 0:1, 0:1, 1:29],
                                         pa4[0:48, :, 13:14, :],
                                         B[0:48, :], 0.0, ALU.add, ALU.max)
            i5 = nc.scalar.activation(A[0:48, 1:2, 15:16, 1:29],
                                      pb4[0:48, :, 0:1, :],
                                      AF.Relu, bias=B[0:48, :])
            i6 = nc.scalar.activation(A[64:112, 1:2, 0:1, 1:29],
                                      pb4[64:112, :, 13:14, :],
                                      AF.Relu, bias=B[64:112, :])
            # PE-W + engine-R same-bank hazard: order every reader after the
            # tile's last matmul (PE completes in program order).
            for rd in (i1, i3, i4):
                add_dep_helper(rd.ins, mA.ins, reason="psum bank PA fully written")
            for rd in (i2a, i2b, i5, i6):
                add_dep_helper(rd.ins, mB.ins, reason="psum bank PB fully written")
            Adst_out[p] = A

        def pool3(p):
            # conv3 psum -> relu (in place) + per-frame accum straight into
            # the Z column for this (timestep, batch).
            PA, PB, mA, mB = P3[p]
            t, j = p // 2, p % 2
            Z = ZT[t % 3]
            ra = nc.scalar.activation(PA[0:112, :], PA[0:112, :],
                                      AF.Relu, bias=B3T[0:112, :],
                                      accum_out=Z[0:112, 2 * j:2 * j + 1])
            rb = nc.scalar.activation(PB[0:112, :], PB[0:112, :],
                                      AF.Relu, bias=B3T[0:112, :],
                                      accum_out=Z[0:112, 2 * j + 1:2 * j + 2])
            add_dep_helper(ra.ins, mA.ins, reason="psum bank PA fully written")
            add_dep_helper(rb.ins, mB.ins, reason="psum bank PB fully written")

        def lstm_step(t):
            # gate layout on psum partitions: f@0:8, i@32:40, o@64:72, g@96:104
            Z = ZT[t % 3]
            Hp, Hn = HT[t % 2], HT[(t + 1) % 2]
            Gf = g_pool.tile([128, 512], f32, name="Gf", tag="gps")
            G = Gf[:, 0:4]
            # K split skips Z rows 48:64 (junk accum from unused psum
            # partitions; could be non-finite at startup)
            nc.tensor.matmul(G[0:104, :], lhsT=WFT[0:48, 0:104], rhs=Z[0:48, :],
                             start=True, stop=False)
            nc.tensor.matmul(G[0:104, :], lhsT=WFT[64:113, 0:104], rhs=Z[64:113, :],
                             start=False, stop=False)
            nc.tensor.matmul(G[0:104, :], lhsT=WHT[0:8, 0:104], rhs=Hp[:, :],
                             start=False, stop=True)
            # each gate activation writes to a base-0 tile (tensor_tensor
            # requires all operands at the same start partition)
            GF = ls_pool.tile([8, 4], f32, name="GF")
            GI = ls_pool.tile([8, 4], f32, name="GI")
            GO = ls_pool.tile([8, 4], f32, name="GO")
            GG = ls_pool.tile([8, 4], f32, name="GG")
            nc.scalar.activation(GF[:, :], G[0:8, :], AF.Sigmoid)
            nc.scalar.activation(GI[:, :], G[32:40, :], AF.Sigmoid)
            nc.scalar.activation(GO[:, :], G[64:72, :], AF.Sigmoid)
            nc.scalar.activation(GG[:, :], G[96:104, :], AF.Tanh)
            Cp, Cn = CT[t % 2], CT[(t + 1) % 2]
            T1 = ls_pool.tile([8, 4], f32, name="T1")
            T2 = ls_pool.tile([8, 4], f32, name="T2")
            nc.vector.tensor_mul(T1[:, :], GF[:, :], Cp[:, :])
            nc.vector.tensor_mul(T2[:, :], GI[:, :], GG[:, :])
            nc.vector.tensor_add(Cn[:, :], T1[:, :], T2[:, :])
            TC = ls_pool.tile([8, 4], f32, name="TC")
            nc.scalar.activation(TC[:, :], Cn[:, :], AF.Tanh)
            nc.vector.tensor_mul(Hn[:, :], GO[:, :], TC[:, :])
            nc.vector.tensor_mul(HST[0:8, 4 * t:4 * t + 4], GO[:, :], TC[:, :])

        # ---- software-pipelined emission ----
        A1 = [None] * _PAIRS
        A2 = [None] * _PAIRS
        for p in range(_PAIRS + 3):
            # pool3 first: its ACTs free the psum banks this step's conv3
            # will reuse (6-buffer rotation), so queue them ahead of the
            # relu stores on the Scalar engine.
            if 3 <= p < _PAIRS + 3:
                pool3(p - 3)
            if p < _PAIRS:
                conv1(p)
            if 1 <= p < _PAIRS + 1:
                q = p - 1
                relu_store(q, P1, B1T, A1B, A1)
                conv23(q, W2T, A1[q], P2)
            if 2 <= p < _PAIRS + 2:
                q = p - 2
                relu_store(q, P2, B2T, A2B, A2)
                conv23(q, W3T, A2[q], P3)
            if p >= 4 and p % 2 == 0:
                lstm_step((p - 4) // 2)

        # ---- output head ----
        Yf = g_pool.tile([128, 512], f32, name="Yf", tag="gps")
        Y = Yf[0:1, 0:128]
        nc.tensor.matmul(Y[:, :], lhsT=WOT[0:8, :], rhs=HST[0:8, :])
        OUTS = cp.tile([1, 128], f32, name="OUTS", tag="OUTS")
        nc.vector.tensor_scalar(OUTS[:, :], Y[:, :], BOT[0:1, :], None, ALU.add)
        nc.sync.dma_start(out_d.ap()[:, :], OUTS[:, :])

    nc.compile()
    return nc


# ---------------------------------------------------------------------------
# Host-side prep
# ---------------------------------------------------------------------------

def _prep_core_inputs(xc, w1, b1, w2, b2, w3, b3, bn1, wf, bf, wi1, bi1, wi2, bi2,
                      wo, bo, bn2, w_out, b_out):
    """xc: [4, 32, 28, 28, 1] float32 for one core. Returns the in_map dict."""
    T, HW = _T, _HW
    xp = np.zeros((_BPC, T, 30, 30), _F32)
    xp[:, :, 1:29, 1:29] = xc[..., 0]

    # im2col for conv1, frame-major block-diagonal: for frame g of pair
    # (t, j) (batch b = 2j+g), rows 0:9 = taps of out rows 0..13, rows 9:18 =
    # taps of out rows 14..27.
    X1 = np.empty((_PAIRS, 2, 18, 392), _F32)
    X1v = X1.reshape(T, 2, 2, 18, 14, 28)  # [t, j, g, krow, r, c]
    for dy in range(3):
        for dx in range(3):
            blkA = xp[:, :, dy:dy + 14, dx:dx + 28]        # [b, t, 14, 28]
            blkB = xp[:, :, 14 + dy:28 + dy, dx:dx + 28]
            for j in range(2):
                for g in range(2):
                    X1v[:, j, g, 3 * dy + dx] = blkA[2 * j + g]
                    X1v[:, j, g, 9 + 3 * dy + dx] = blkB[2 * j + g]

    def wpack1():
        # block diagonal. Frame 0 (K rows 0:18): chunk A (out rows 0..13) ->
        # out cols 0:48, chunk B -> cols 64:112. Frame 1 (K rows 64:82) is
        # SWAPPED to match the conv23 PB tile convention: chunk A -> cols
        # 64:112, chunk B -> cols 0:48.
        w = np.zeros((128, 128), _F32)
        w9 = w1.reshape(9, _F)
        w[0:9, 0:48] = w9
        w[9:18, 64:112] = w9
        w[64:73, 64:112] = w9
        w[73:82, 0:48] = w9
        return w

    def wpack(wn):
        w = np.zeros((128, 9, 64), _F32)
        for tap in range(9):
            m = wn[tap // 3, tap % 3]  # [48, 48]
            w[0:48, tap, 0:48] = m
            w[64:112, tap, 0:48] = m
        return w

    def bpack(bn):
        b = np.zeros((128, 1), _F32)
        b[0:48, 0] = bn
        b[64:112, 0] = bn
        return b

    bn1_g, bn1_b, bn1_m, bn1_v = bn1
    bn2_g, bn2_b, bn2_m, bn2_v = bn2
    s1 = bn1_g / np.sqrt(bn1_v + _EPS)
    t1 = bn1_b - bn1_m * s1
    Wx = np.concatenate([wf[:_F], wi1[:_F], wo[:_F], wi2[:_F]], axis=1)  # [48, 32] f,i,o,g
    Wh = np.concatenate([wf[_F:], wi1[_F:], wo[_F:], wi2[_F:]], axis=1)  # [8, 32]
    bias = np.concatenate([bf, bi1, bo, bi2]) + t1 @ Wx                  # [32]
    Wxs = (s1[:, None] * Wx) / float(HW * HW)

    def spread(m):
        # [r, 32] gate-packed (f,i,o,g x 8) -> [r, 128] at cols f@0:8, i@32:40,
        # o@64:72, g@96:104
        out = np.zeros((m.shape[0], 128), _F32)
        for gidx in range(4):
            out[:, 32 * gidx:32 * gidx + 8] = m[:, 8 * gidx:8 * gidx + 8]
        return out

    WF = np.zeros((128, 128), _F32)
    WF[0:48] = spread(Wxs)
    WF[64:112] = spread(Wxs)
    WF[112] = spread(bias[None, :])[0]     # Z row 112 is the ones row
    WH = np.zeros((128, 128), _F32)
    WH[0:8] = spread(Wh)

    s2 = bn2_g / np.sqrt(bn2_v + _EPS)
    t2 = bn2_b - bn2_m * s2
    WO = np.zeros((128, 1), _F32)
    WO[0:8, 0] = s2 * w_out[:, 0]
    bot = np.array([[t2 @ w_out[:, 0] + b_out[0]]], _F32)

    return {
        "x1": X1.astype(_BF16),
        "w1t": wpack1().astype(_BF16),
        "w2t": wpack(w2).astype(_BF16), "w3t": wpack(w3).astype(_BF16),
        "b1t": bpack(b1), "b2t": bpack(b2), "b3t": bpack(b3),
        "wft": WF, "wht": WH, "woutt": WO, "bot": bot,
    }


_PROG = None
_LAST_RESULTS = None


def _install_ntff_hook():
    """The agent image's antenv lacks axon_hooks; synthesize it and register
    the ctypes-based NTFF profile hook from trn_agent_boot."""
    import types
    import antenv
    if getattr(antenv, "axon_hooks", None) is not None:
        return
    m = types.ModuleType("antenv.axon_hooks")
    state = {"h": None}
    m.set_axon_ntff_profile_hook = lambda h: state.__setitem__("h", h)
    m.get_axon_ntff_profile_hook = lambda: state["h"]
    sys.modules["antenv.axon_hooks"] = m
    antenv.axon_hooks = m
    try:
        from trn_agent_boot.trn_boot import _ntff_profile_via_ctypes
        m.set_axon_ntff_profile_hook(_ntff_profile_via_ctypes("/opt/axon/libaxon_pjrt.so"))
    except Exception as e:
        print("ntff hook install failed:", e)


def kernel(**inputs):
    global _PROG
    inp = {k: np.asarray(v, dtype=np.asarray(v).dtype) for k, v in inputs.items()}
    x = np.asarray(inp["x"], _F32)
    w2 = np.asarray(inp["w2"], _F32)
    w3 = np.asarray(inp["w3"], _F32)
    bn1 = tuple(np.asarray(inp[k], _F32) for k in ("bn1_g", "bn1_b", "bn1_m", "bn1_v"))
    bn2 = tuple(np.asarray(inp[k], _F32) for k in ("bn2_g", "bn2_b", "bn2_m", "bn2_v"))

    in_maps = []
    for c in range(_NCORES):
        xc = x[c * _BPC:(c + 1) * _BPC]
        in_maps.append(_prep_core_inputs(
            xc, np.asarray(inp["w1"], _F32), np.asarray(inp["b1"], _F32),
            w2, np.asarray(inp["b2"], _F32), w3, np.asarray(inp["b3"], _F32),
            bn1,
            np.asarray(inp["wf"], _F32), np.asarray(inp["bf"], _F32),
            np.asarray(inp["wi1"], _F32), np.asarray(inp["bi1"], _F32),
            np.asarray(inp["wi2"], _F32), np.asarray(inp["bi2"], _F32),
            np.asarray(inp["wo"], _F32), np.asarray(inp["bo"], _F32),
            bn2, np.asarray(inp["w_out"], _F32), np.asarray(inp["b_out"], _F32),
        ))

    if _PROG is None:
        _PROG = _build_program()
    from concourse.bass_utils import run_bass_kernel_spmd
    import os as _os
    if _os.environ.get("TRN_KERNEL_TRACE"):
        _install_ntff_hook()
    res = run_bass_kernel_spmd(_PROG, in_maps, core_ids=list(range(_NCORES)),
                               trace=bool(_os.environ.get("TRN_KERNEL_TRACE")))
    global _LAST_RESULTS
    _LAST_RESULTS = res

    out = np.empty((_B, _T, 1), _F32)
    for c in range(_NCORES):
        yc = res.results[c]["out"].reshape(_T, _BPC).T  # [4, 32]
        out[c * _BPC:(c + 1) * _BPC, :, 0] = yc
    return out


if __name__ == "__main__":
    pass


# revision 40
# speedup vs baseline: 1.0018x; 1.0018x over previous
"""Trainium2 Bass kernel for BasicCNN_LSTM (3x conv3x3+relu -> BN -> GAP -> LSTM -> BN -> dense).

Sharding: data-parallel over batch across 8 NeuronCores (4 batches/core).

Per-core plan (128 frames = 4 batches x 32 timesteps, processed as 64 frame-pairs):
  - conv1 (C=1 -> 48): host-built im2col [18, N] block-diagonal matmul (two 14-row
    chunks of one frame per column stream).
  - conv2/conv3 (48 -> 48): 9 tap-accumulated bf16 matmuls (K=48), 4 PE quadrant
    streams per tap; activations stored split: pixel-row-half 0 at SBUF partitions
    0:48, half 1 at partitions 64:112, each padded [16, 30] bf16.
  - PSUM banks are frame-major: bank PA = frame 0 (rows 0..13 at psum partitions
    0:64 from half0, rows 14..27 at 64:128 from half1), bank PB = frame 1. This
    makes the psum->act-buffer store a single [112]-partition op per frame plus
    two 28-col seam slivers, and GAP a single accum ACT per frame.
  - conv bias via activation bias operand; BN1 folded into LSTM input weights;
    GAP via activation accum_out written directly into Z columns; /784 in weights.
  - LSTM: one K=113 matmul per step (rhs Z = [112 pooled sums; ones row 112]);
    sigmoid/tanh on ACT, elementwise on DVE. BN2 + output dense folded into a
    final K=9 matmul over the stored h sequence.
"""

import sys

sys.path.insert(0, "/opt/trn_rl_repo")

import numpy as np
import ml_dtypes

_BF16 = ml_dtypes.bfloat16
_NCORES = 8
_B, _T, _HW, _F, _U = 32, 32, 28, 48, 8
_EPS = 1e-3
_BPC = _B // _NCORES          # batches per core (4)
_PAIRS = _BPC * _T // 2       # frame pairs per core (64)

_F32 = np.float32


# ---------------------------------------------------------------------------
# Device program
# ---------------------------------------------------------------------------

def _build_program():
    import concourse.bass as bass  # noqa: F401
    import concourse.tile as tile
    from concourse.tile import add_dep_helper
    from concourse import bacc, mybir

    f32 = mybir.dt.float32
    bf16 = mybir.dt.bfloat16
    AF = mybir.ActivationFunctionType
    ALU = mybir.AluOpType

    nc = bacc.Bacc("TRN2", target_bir_lowering=False, debug=False, num_devices=_NCORES)

    # DRAM I/O
    x1_d = nc.dram_tensor("x1", (_PAIRS, 2, 18, 392), bf16, kind="ExternalInput")
    w1_d = nc.dram_tensor("w1t", (128, 128), bf16, kind="ExternalInput")
    w2_d = nc.dram_tensor("w2t", (128, 9, 64), bf16, kind="ExternalInput")
    w3_d = nc.dram_tensor("w3t", (128, 9, 64), bf16, kind="ExternalInput")
    b1_d = nc.dram_tensor("b1t", (128, 1), f32, kind="ExternalInput")
    b2_d = nc.dram_tensor("b2t", (128, 1), f32, kind="ExternalInput")
    b3_d = nc.dram_tensor("b3t", (128, 1), f32, kind="ExternalInput")
    wf_d = nc.dram_tensor("wft", (128, 128), f32, kind="ExternalInput")
    wh_d = nc.dram_tensor("wht", (128, 128), f32, kind="ExternalInput")
    bo_d = nc.dram_tensor("bot", (1, 1), f32, kind="ExternalInput")
    wo_d = nc.dram_tensor("woutt", (128, 1), f32, kind="ExternalInput")
    out_d = nc.dram_tensor("out", (1, 128), f32, kind="ExternalOutput")

    from contextlib import ExitStack
    with tile.TileContext(nc) as tc, ExitStack() as ctx:
        # ---- persistent tiles (one const pool, unique tag per tile) ----
        cp = ctx.enter_context(tc.tile_pool(name="const", bufs=1))
        W1T = cp.tile([128, 128], bf16, name="W1T", tag="W1T")
        W2T = cp.tile([128, 9, 64], bf16, name="W2T", tag="W2T")
        W3T = cp.tile([128, 9, 64], bf16, name="W3T", tag="W3T")
        B1T = cp.tile([128, 1], f32, name="B1T", tag="B1T")
        B2T = cp.tile([128, 1], f32, name="B2T", tag="B2T")
        B3T = cp.tile([128, 1], f32, name="B3T", tag="B3T")
        WFT = cp.tile([128, 128], f32, name="WFT", tag="WFT")
        WHT = cp.tile([128, 128], f32, name="WHT", tag="WHT")
        BOT = cp.tile([1, 1], f32, name="BOT", tag="BOT")
        WOT = cp.tile([128, 1], f32, name="WOT", tag="WOT")
        HST = cp.tile([128, 128], f32, name="HST", tag="HST")   # rows 0:8 h, col = 4t+b
        ZT = [cp.tile([128, 4], f32, name=f"ZT{i}", tag=f"ZT{i}") for i in range(3)]
        CT = [cp.tile([8, 4], f32, name=f"CT{i}", tag=f"CT{i}") for i in range(2)]
        HT = [cp.tile([8, 4], f32, name=f"HT{i}", tag=f"HT{i}") for i in range(2)]
        # persistent split act buffers (pads zeroed once)
        A1B = [cp.tile([128, 2, 16, 30], bf16, name=f"A1B{i}", tag=f"A1B{i}")
               for i in range(3)]
        A2B = [cp.tile([128, 2, 16, 30], bf16, name=f"A2B{i}", tag=f"A2B{i}")
               for i in range(3)]

        nc.sync.dma_start(W1T[:, :], w1_d.ap()[:, :])
        nc.gpsimd.dma_start(W2T[:, :, :], w2_d.ap()[:, :, :])
        nc.scalar.dma_start(W3T[:, :, :], w3_d.ap()[:, :, :])
        nc.sync.dma_start(B1T[:, :], b1_d.ap()[:, :])
        nc.gpsimd.dma_start(B2T[:, :], b2_d.ap()[:, :])
        nc.scalar.dma_start(B3T[:, :], b3_d.ap()[:, :])
        nc.gpsimd.dma_start(WFT[:, :], wf_d.ap()[:, :])
        nc.gpsimd.dma_start(WHT[:, :], wh_d.ap()[:, :])
        nc.scalar.dma_start(WOT[:, :], wo_d.ap()[:, :])
        nc.scalar.dma_start(BOT[:, :], bo_d.ap()[:, :])

        # Z rows 96:128 := 1.0 once (row 112 is the LSTM bias-ones row; rows
        # 0:112 are fully overwritten by pool accum_out before every read;
        # base partition must be a multiple of 32)
        for z in ZT:
            nc.vector.memset(z[96:128, :], 1.0)
        nc.vector.memset(CT[0][:, :], 0.0)
        nc.vector.memset(HT[0][:, :], 0.0)

        # one-time pad zeroing of the persistent act buffers:
        # half0 (p0:48): pad row 0; half1 (p64:112): pad row 15; cols 0, 29 both.
        for A in A1B + A2B:
            nc.gpsimd.memset(A[0:48, :, 0:1, :], 0.0)
            nc.gpsimd.memset(A[64:112, :, 15:16, :], 0.0)
            nc.gpsimd.memset(A[0:128, :, :, 0:1], 0.0)
            nc.gpsimd.memset(A[0:128, :, :, 29:30], 0.0)

        # ---- pools ----
        x1_pool = ctx.enter_context(tc.tile_pool(name="x1p", bufs=6))
        ps_pool = ctx.enter_context(tc.tile_pool(name="psp", bufs=6, space="PSUM"))
        g_pool = ctx.enter_context(tc.tile_pool(name="gp", bufs=2, space="PSUM"))
        ls_pool = ctx.enter_context(tc.tile_pool(name="lsp", bufs=4))

        # per-pair state carried between loop stages
        P1 = [None] * _PAIRS   # (PA, PB, mA, mB): bank = frame in pair
        P2 = [None] * _PAIRS
        P3 = [None] * _PAIRS

        def conv1(p):
            # block-diagonal K=18/M=112 packing: each streamed column carries
            # the 9 im2col taps of one frame; chunk rows 0..13 at lhsT rows
            # 0:9 -> psum 0:48, chunk rows 14..27 at rows 9:18 -> psum 64:112.
            # Bank PA = frame 0 of the pair, PB = frame 1.
            X = x1_pool.tile([128, 392], bf16, name="X")
            nc.sync.dma_start(X[0:18, :], x1_d.ap()[p, 0])
            nc.sync.dma_start(X[64:82, :], x1_d.ap()[p, 1])
            PAf = ps_pool.tile([128, 512], f32, name="PAf", tag="cps")
            PBf = ps_pool.tile([128, 512], f32, name="PBf", tag="cps")
            PA, PB = PAf[:, 0:392], PBf[:, 0:392]
            mA = nc.tensor.matmul(PA[0:112, :], lhsT=W1T[0:18, 0:112], rhs=X[0:18, :],
                                  skip_group_check=True)
            mB = nc.tensor.matmul(PB[0:112, :], lhsT=W1T[64:82, 0:112], rhs=X[64:82, :],
                                  skip_group_check=True)
            P1[p] = (PA, PB, mA, mB)

        def conv23(p, W, A, Pout):
            # bank = frame g. PA (frame 0): block 0:64 <- half0 (out rows
            # 0..13, lo), block 64:128 <- half1 (rows 14..27, hi). PB (frame
            # 1) is SWAPPED: block 0:64 <- half1 (rows 14..27, hi), block
            # 64:128 <- half0 (rows 0..13, lo) -- so the four matmuls of a
            # tap land on four distinct PE tiles (no same-tile back-to-back
            # streams, LDWEIGHTS stays in the background buffer).
            PAf = ps_pool.tile([128, 512], f32, name="PA23f", tag="cps")
            PBf = ps_pool.tile([128, 512], f32, name="PB23f", tag="cps")
            PA, PB = PAf[:, 0:392], PBf[:, 0:392]
            # dy=1 taps first: their windows (buf rows 1..14) only touch the
            # big relu_store writes, not the seam rows 0/15 -- so this pair's
            # first matmuls don't wait on the DVE seam chain.
            for ti, tap in enumerate((3, 4, 5, 0, 1, 2, 6, 7, 8)):
                dy, dx = tap // 3, tap % 3
                st = ti == 0
                sp = ti == 8
                lo = W[0:48, tap, :]
                hi = W[64:112, tap, :]
                # slot 1: tiles (0,0) + (64,64); slot 2: tiles (0,64) + (64,0)
                nc.tensor.matmul(PA[0:64, :], lhsT=lo,
                                 rhs=A[0:48, 0:1, dy:dy + 14, dx:dx + 28],
                                 start=st, stop=sp, skip_group_check=True)
                mA = nc.tensor.matmul(PA[64:128, :], lhsT=hi,
                                 rhs=A[64:112, 0:1, dy:dy + 14, dx:dx + 28],
                                 start=st, stop=sp, skip_group_check=True)
                mB = nc.tensor.matmul(PB[64:128, :], lhsT=lo,
                                 rhs=A[0:48, 1:2, dy:dy + 14, dx:dx + 28],
                                 start=st, stop=sp, skip_group_check=True)
                nc.tensor.matmul(PB[0:64, :], lhsT=hi,
                                 rhs=A[64:112, 1:2, dy:dy + 14, dx:dx + 28],
                                 start=st, stop=sp, skip_group_check=True)
            Pout[p] = (PA, PB, mA, mB)

        def relu_store(p, Psrc, B, Abufs, Adst_out):
            # psum (frame-banked) -> padded split act buffer, relu + bias.
            PA, PB, mA, mB = Psrc[p]
            A = Abufs[p % 3]
            # big write: whole frame, both halves in one [112]-partition op.
            # buf0 rows 1..14 <- out rows 0..13 (block 0:64); buf1 rows 1..14
            # <- out rows 14..27 (block 64:128); junk lands on partitions
            # 48:64 (unused gap).
            pa4 = PA.rearrange("p (a r c) -> p a r c", a=1, r=14, c=28)
            pb4 = PB.rearrange("p (a r c) -> p a r c", a=1, r=14, c=28)
            # frame 0 (PA, straight): one [112]-partition write covers both
            # halves (buf rows 1..14 in each); junk lands on partitions 48:64.
            i1 = nc.scalar.activation(A[0:112, 0:1, 1:15, 1:29], pa4[0:112, :, :, :],
                                      AF.Relu, bias=B[0:112, :])
            # frame 1 (PB, swapped blocks): two crossed half-writes. Both are
            # partition-base-shifted (in base != out base), which only the
            # DVE handles; the ACT takes the base-aligned seam slivers.
            i2a = nc.vector.tensor_scalar(A[0:48, 1:2, 1:15, 1:29],
                                          pb4[64:112, :, :, :],
                                          B[64:112, :], 0.0, ALU.add, ALU.max)
            i2b = nc.vector.tensor_scalar(A[64:112, 1:2, 1:15, 1:29],
                                          pb4[0:48, :, :, :],
                                          B[0:48, :], 0.0, ALU.add, ALU.max)
            # seam slivers: buf0 row 15 = out row 14 (first 28 cols of the
            # rows-14..27 block); buf1 row 0 = out row 13 (last 28 cols of
            # the rows-0..13 block).
            i3 = nc.vector.tensor_scalar(A[0:48, 0:1, 15:16, 1:29],
                                         pa4[64:112, :, 0:1, :],
                                         B[64:112, :], 0.0, ALU.add, ALU.max)
            i4 = nc.vector.tensor_scalar(A[64:112, 0:1, 0:1, 1:29],
                                         pa4[0:48, :, 13:14, :],
                                         B[0:48, :], 0.0, ALU.add, ALU.max)
            i5 = nc.scalar.activation(A[0:48, 1:2, 15:16, 1:29],
                                      pb4[0:48, :, 0:1, :],
                                      AF.Relu, bias=B[0:48, :])
            i6 = nc.scalar.activation(A[64:112, 1:2, 0:1, 1:29],
                                      pb4[64:112, :, 13:14, :],
                                      AF.Relu, bias=B[64:112, :])
            # PE-W + engine-R same-bank hazard: order every reader after the
            # tile's last matmul (PE completes in program order).
            for rd in (i1, i3, i4):
                add_dep_helper(rd.ins, mA.ins, reason="psum bank PA fully written")
            for rd in (i2a, i2b, i5, i6):
                add_dep_helper(rd.ins, mB.ins, reason="psum bank PB fully written")
            Adst_out[p] = A

        def pool3(p):
            # conv3 psum -> relu (in place) + per-frame accum straight into
            # the Z column for this (timestep, batch).
            PA, PB, mA, mB = P3[p]
            t, j = p // 2, p % 2
            Z = ZT[t % 3]
            ra = nc.scalar.activation(PA[0:112, :], PA[0:112, :],
                                      AF.Relu, bias=B3T[0:112, :],
                                      accum_out=Z[0:112, 2 * j:2 * j + 1])
            rb = nc.scalar.activation(PB[0:112, :], PB[0:112, :],
                                      AF.Relu, bias=B3T[0:112, :],
                                      accum_out=Z[0:112, 2 * j + 1:2 * j + 2])
            add_dep_helper(ra.ins, mA.ins, reason="psum bank PA fully written")
            add_dep_helper(rb.ins, mB.ins, reason="psum bank PB fully written")

        def lstm_step(t):
            # gate layout on psum partitions: f@0:8, i@32:40, o@64:72, g@96:104
            Z = ZT[t % 3]
            Hp, Hn = HT[t % 2], HT[(t + 1) % 2]
            Gf = g_pool.tile([128, 512], f32, name="Gf", tag="gps")
            G = Gf[:, 0:4]
            # K split skips Z rows 48:64 (junk accum from unused psum
            # partitions; could be non-finite at startup)
            nc.tensor.matmul(G[0:104, :], lhsT=WFT[0:48, 0:104], rhs=Z[0:48, :],
                             start=True, stop=False)
            nc.tensor.matmul(G[0:104, :], lhsT=WFT[64:113, 0:104], rhs=Z[64:113, :],
                             start=False, stop=False)
            nc.tensor.matmul(G[0:104, :], lhsT=WHT[0:8, 0:104], rhs=Hp[:, :],
                             start=False, stop=True)
            # each gate activation writes to a base-0 tile (tensor_tensor
            # requires all operands at the same start partition)
            GF = ls_pool.tile([8, 4], f32, name="GF")
            GI = ls_pool.tile([8, 4], f32, name="GI")
            GO = ls_pool.tile([8, 4], f32, name="GO")
            GG = ls_pool.tile([8, 4], f32, name="GG")
            nc.scalar.activation(GF[:, :], G[0:8, :], AF.Sigmoid)
            nc.scalar.activation(GI[:, :], G[32:40, :], AF.Sigmoid)
            nc.scalar.activation(GO[:, :], G[64:72, :], AF.Sigmoid)
            nc.scalar.activation(GG[:, :], G[96:104, :], AF.Tanh)
            Cp, Cn = CT[t % 2], CT[(t + 1) % 2]
            T1 = ls_pool.tile([8, 4], f32, name="T1")
            T2 = ls_pool.tile([8, 4], f32, name="T2")
            nc.vector.tensor_mul(T1[:, :], GF[:, :], Cp[:, :])
            nc.vector.tensor_mul(T2[:, :], GI[:, :], GG[:, :])
            nc.vector.tensor_add(Cn[:, :], T1[:, :], T2[:, :])
            TC = ls_pool.tile([8, 4], f32, name="TC")
            nc.scalar.activation(TC[:, :], Cn[:, :], AF.Tanh)
            nc.vector.tensor_mul(Hn[:, :], GO[:, :], TC[:, :])
            nc.vector.tensor_mul(HST[0:8, 4 * t:4 * t + 4], GO[:, :], TC[:, :])

        # ---- software-pipelined emission ----
        A1 = [None] * _PAIRS
        A2 = [None] * _PAIRS
        for p in range(_PAIRS + 3):
            # pool3 first: its ACTs free the psum banks this step's conv3
            # will reuse (6-buffer rotation), so queue them ahead of the
            # relu stores on the Scalar engine.
            if 3 <= p < _PAIRS + 3:
                pool3(p - 3)
            if p < _PAIRS:
                conv1(p)
            if 1 <= p < _PAIRS + 1:
                q = p - 1
                relu_store(q, P1, B1T, A1B, A1)
                conv23(q, W2T, A1[q], P2)
            if 2 <= p < _PAIRS + 2:
                q = p - 2
                relu_store(q, P2, B2T, A2B, A2)
                conv23(q, W3T, A2[q], P3)
            if p >= 4 and p % 2 == 0:
                lstm_step((p - 4) // 2)

        # ---- output head ----
        Yf = g_pool.tile([128, 512], f32, name="Yf", tag="gps")
        Y = Yf[0:1, 0:128]
        nc.tensor.matmul(Y[:, :], lhsT=WOT[0:8, :], rhs=HST[0:8, :])
        OUTS = cp.tile([1, 128], f32, name="OUTS", tag="OUTS")
        nc.vector.tensor_scalar(OUTS[:, :], Y[:, :], BOT[0:1, :], None, ALU.add)
        nc.sync.dma_start(out_d.ap()[:, :], OUTS[:, :])

    nc.compile()
    return nc


# ---------------------------------------------------------------------------
# Host-side prep
# ---------------------------------------------------------------------------

def _prep_core_inputs(xc, w1, b1, w2, b2, w3, b3, bn1, wf, bf, wi1, bi1, wi2, bi2,
                      wo, bo, bn2, w_out, b_out):
    """xc: [4, 32, 28, 28, 1] float32 for one core. Returns the in_map dict."""
    T, HW = _T, _HW
    xp = np.zeros((_BPC, T, 30, 30), _F32)
    xp[:, :, 1:29, 1:29] = xc[..., 0]

    # im2col for conv1, frame-major block-diagonal: for frame g of pair
    # (t, j) (batch b = 2j+g), rows 0:9 = taps of out rows 0..13, rows 9:18 =
    # taps of out rows 14..27.
    X1 = np.empty((_PAIRS, 2, 18, 392), _F32)
    X1v = X1.reshape(T, 2, 2, 18, 14, 28)  # [t, j, g, krow, r, c]
    for dy in range(3):
        for dx in range(3):
            blkA = xp[:, :, dy:dy + 14, dx:dx + 28]        # [b, t, 14, 28]
            blkB = xp[:, :, 14 + dy:28 + dy, dx:dx + 28]
            for j in range(2):
                for g in range(2):
                    X1v[:, j, g, 3 * dy + dx] = blkA[2 * j + g]
                    X1v[:, j, g, 9 + 3 * dy + dx] = blkB[2 * j + g]

    def wpack1():
        # block diagonal. Frame 0 (K rows 0:18): chunk A (out rows 0..13) ->
        # out cols 0:48, chunk B -> cols 64:112. Frame 1 (K rows 64:82) is
        # SWAPPED to match the conv23 PB tile convention: chunk A -> cols
        # 64:112, chunk B -> cols 0:48.
        w = np.zeros((128, 128), _F32)
        w9 = w1.reshape(9, _F)
        w[0:9, 0:48] = w9
        w[9:18, 64:112] = w9
        w[64:73, 64:112] = w9
        w[73:82, 0:48] = w9
        return w

    def wpack(wn):
        w = np.zeros((128, 9, 64), _F32)
        for tap in range(9):
            m = wn[tap // 3, tap % 3]  # [48, 48]
            w[0:48, tap, 0:48] = m
            w[64:112, tap, 0:48] = m
        return w

    def bpack(bn):
        b = np.zeros((128, 1), _F32)
        b[0:48, 0] = bn
        b[64:112, 0] = bn
        return b

    bn1_g, bn1_b, bn1_m, bn1_v = bn1
    bn2_g, bn2_b, bn2_m, bn2_v = bn2
    s1 = bn1_g / np.sqrt(bn1_v + _EPS)
    t1 = bn1_b - bn1_m * s1
    Wx = np.concatenate([wf[:_F], wi1[:_F], wo[:_F], wi2[:_F]], axis=1)  # [48, 32] f,i,o,g
    Wh = np.concatenate([wf[_F:], wi1[_F:], wo[_F:], wi2[_F:]], axis=1)  # [8, 32]
    bias = np.concatenate([bf, bi1, bo, bi2]) + t1 @ Wx                  # [32]
    Wxs = (s1[:, None] * Wx) / float(HW * HW)

    def spread(m):
        # [r, 32] gate-packed (f,i,o,g x 8) -> [r, 128] at cols f@0:8, i@32:40,
        # o@64:72, g@96:104
        out = np.zeros((m.shape[0], 128), _F32)
        for gidx in range(4):
            out[:, 32 * gidx:32 * gidx + 8] = m[:, 8 * gidx:8 * gidx + 8]
        return out

    WF = np.zeros((128, 128), _F32)
    WF[0:48] = spread(Wxs)
    WF[64:112] = spread(Wxs)
    WF[112] = spread(bias[None, :])[0]     # Z row 112 is the ones row
    WH = np.zeros((128, 128), _F32)
    WH[0:8] = spread(Wh)

    s2 = bn2_g / np.sqrt(bn2_v + _EPS)
    t2 = bn2_b - bn2_m * s2
    WO = np.zeros((128, 1), _F32)
    WO[0:8, 0] = s2 * w_out[:, 0]
    bot = np.array([[t2 @ w_out[:, 0] + b_out[0]]], _F32)

    return {
        "x1": X1.astype(_BF16),
        "w1t": wpack1().astype(_BF16),
        "w2t": wpack(w2).astype(_BF16), "w3t": wpack(w3).astype(_BF16),
        "b1t": bpack(b1), "b2t": bpack(b2), "b3t": bpack(b3),
        "wft": WF, "wht": WH, "woutt": WO, "bot": bot,
    }


_PROG = None
_LAST_RESULTS = None


def _install_ntff_hook():
    """The agent image's antenv lacks axon_hooks; synthesize it and register
    the ctypes-based NTFF profile hook from trn_agent_boot."""
    import types
    import antenv
    if getattr(antenv, "axon_hooks", None) is not None:
        return
    m = types.ModuleType("antenv.axon_hooks")
    state = {"h": None}
    m.set_axon_ntff_profile_hook = lambda h: state.__setitem__("h", h)
    m.get_axon_ntff_profile_hook = lambda: state["h"]
    sys.modules["antenv.axon_hooks"] = m
    antenv.axon_hooks = m
    try:
        from trn_agent_boot.trn_boot import _ntff_profile_via_ctypes
        m.set_axon_ntff_profile_hook(_ntff_profile_via_ctypes("/opt/axon/libaxon_pjrt.so"))
    except Exception as e:
        print("ntff hook install failed:", e)


def kernel(**inputs):
    global _PROG
    inp = {k: np.asarray(v, dtype=np.asarray(v).dtype) for k, v in inputs.items()}
    x = np.asarray(inp["x"], _F32)
    w2 = np.asarray(inp["w2"], _F32)
    w3 = np.asarray(inp["w3"], _F32)
    bn1 = tuple(np.asarray(inp[k], _F32) for k in ("bn1_g", "bn1_b", "bn1_m", "bn1_v"))
    bn2 = tuple(np.asarray(inp[k], _F32) for k in ("bn2_g", "bn2_b", "bn2_m", "bn2_v"))

    in_maps = []
    for c in range(_NCORES):
        xc = x[c * _BPC:(c + 1) * _BPC]
        in_maps.append(_prep_core_inputs(
            xc, np.asarray(inp["w1"], _F32), np.asarray(inp["b1"], _F32),
            w2, np.asarray(inp["b2"], _F32), w3, np.asarray(inp["b3"], _F32),
            bn1,
            np.asarray(inp["wf"], _F32), np.asarray(inp["bf"], _F32),
            np.asarray(inp["wi1"], _F32), np.asarray(inp["bi1"], _F32),
            np.asarray(inp["wi2"], _F32), np.asarray(inp["bi2"], _F32),
            np.asarray(inp["wo"], _F32), np.asarray(inp["bo"], _F32),
            bn2, np.asarray(inp["w_out"], _F32), np.asarray(inp["b_out"], _F32),
        ))

    if _PROG is None:
        _PROG = _build_program()
    from concourse.bass_utils import run_bass_kernel_spmd
    import os as _os
    if _os.environ.get("TRN_KERNEL_TRACE"):
        _install_ntff_hook()
    res = run_bass_kernel_spmd(_PROG, in_maps, core_ids=list(range(_NCORES)),
                               trace=bool(_os.environ.get("TRN_KERNEL_TRACE")))
    global _LAST_RESULTS
    _LAST_RESULTS = res

    out = np.empty((_B, _T, 1), _F32)
    for c in range(_NCORES):
        yc = res.results[c]["out"].reshape(_T, _BPC).T  # [4, 32]
        out[c * _BPC:(c + 1) * _BPC, :, 0] = yc
    return out


if __name__ == "__main__":
    pass


# revision 44
# speedup vs baseline: 1.1419x; 1.1398x over previous
"""Trainium2 Bass kernel for BasicCNN_LSTM (3x conv3x3+relu -> BN -> GAP -> LSTM -> BN -> dense).

Sharding: data-parallel over batch across 8 NeuronCores (4 batches/core).

Per-core plan (128 frames = 4 batches x 32 timesteps, processed as 64 frame-pairs):
  - conv1 (C=1 -> 48): host-built im2col [18, N] block-diagonal matmul (two 14-row
    chunks of one frame per column stream).
  - conv2/conv3 (48 -> 48): 9 tap-accumulated bf16 matmuls (K=48), 4 PE quadrant
    streams per tap; activations stored split: pixel-row-half 0 at SBUF partitions
    0:48, half 1 at partitions 64:112, each padded [16, 30] bf16.
  - PSUM banks are frame-major: bank PA = frame 0 (rows 0..13 at psum partitions
    0:64 from half0, rows 14..27 at 64:128 from half1), bank PB = frame 1. This
    makes the psum->act-buffer store a single [112]-partition op per frame plus
    two 28-col seam slivers, and GAP a single accum ACT per frame.
  - conv bias via activation bias operand; BN1 folded into LSTM input weights;
    GAP via activation accum_out written directly into Z columns; /784 in weights.
  - LSTM: one K=113 matmul per step (rhs Z = [112 pooled sums; ones row 112]);
    sigmoid/tanh on ACT, elementwise on DVE. BN2 + output dense folded into a
    final K=9 matmul over the stored h sequence.
"""

import sys

sys.path.insert(0, "/opt/trn_rl_repo")

import numpy as np
import ml_dtypes

_BF16 = ml_dtypes.bfloat16
_NCORES = 8
_B, _T, _HW, _F, _U = 32, 32, 28, 48, 8
_EPS = 1e-3
_BPC = _B // _NCORES          # batches per core (4)
_PAIRS = _BPC * _T // 2       # frame pairs per core (64)

_F32 = np.float32


# ---------------------------------------------------------------------------
# Device program
# ---------------------------------------------------------------------------

def _build_program():
    import concourse.bass as bass  # noqa: F401
    import concourse.tile as tile
    from concourse.tile import add_dep_helper
    from concourse import bacc, mybir

    f32 = mybir.dt.float32
    bf16 = mybir.dt.bfloat16
    AF = mybir.ActivationFunctionType
    ALU = mybir.AluOpType

    nc = bacc.Bacc("TRN2", target_bir_lowering=False, debug=False, num_devices=_NCORES)

    # DRAM I/O
    x1_d = nc.dram_tensor("x1", (_PAIRS, 2, 18, 392), bf16, kind="ExternalInput")
    w1_d = nc.dram_tensor("w1t", (128, 128), bf16, kind="ExternalInput")
    w2_d = nc.dram_tensor("w2t", (128, 9, 64), bf16, kind="ExternalInput")
    w3_d = nc.dram_tensor("w3t", (128, 9, 64), bf16, kind="ExternalInput")
    b1_d = nc.dram_tensor("b1t", (128, 1), f32, kind="ExternalInput")
    b2_d = nc.dram_tensor("b2t", (128, 1), f32, kind="ExternalInput")
    b3_d = nc.dram_tensor("b3t", (128, 1), f32, kind="ExternalInput")
    wf_d = nc.dram_tensor("wft", (128, 128), f32, kind="ExternalInput")
    wh_d = nc.dram_tensor("wht", (128, 128), f32, kind="ExternalInput")
    bo_d = nc.dram_tensor("bot", (1, 1), f32, kind="ExternalInput")
    wo_d = nc.dram_tensor("woutt", (128, 1), f32, kind="ExternalInput")
    out_d = nc.dram_tensor("out", (1, 128), f32, kind="ExternalOutput")

    from contextlib import ExitStack
    with tile.TileContext(nc) as tc, ExitStack() as ctx:
        # ---- persistent tiles (one const pool, unique tag per tile) ----
        cp = ctx.enter_context(tc.tile_pool(name="const", bufs=1))
        W1T = cp.tile([128, 128], bf16, name="W1T", tag="W1T")
        W2T = cp.tile([128, 9, 64], bf16, name="W2T", tag="W2T")
        W3T = cp.tile([128, 9, 64], bf16, name="W3T", tag="W3T")
        B1T = cp.tile([128, 1], f32, name="B1T", tag="B1T")
        B2T = cp.tile([128, 1], f32, name="B2T", tag="B2T")
        B3T = cp.tile([128, 1], f32, name="B3T", tag="B3T")
        WFT = cp.tile([128, 128], f32, name="WFT", tag="WFT")
        WHT = cp.tile([128, 128], f32, name="WHT", tag="WHT")
        BOT = cp.tile([1, 1], f32, name="BOT", tag="BOT")
        WOT = cp.tile([128, 1], f32, name="WOT", tag="WOT")
        HST = cp.tile([128, 128], f32, name="HST", tag="HST")   # rows 0:8 h, col = 4t+b
        ZT = [cp.tile([128, 4], f32, name=f"ZT{i}", tag=f"ZT{i}") for i in range(3)]
        CT = [cp.tile([8, 4], f32, name=f"CT{i}", tag=f"CT{i}") for i in range(2)]
        HT = [cp.tile([8, 4], f32, name=f"HT{i}", tag=f"HT{i}") for i in range(2)]
        # persistent split act buffers (pads zeroed once)
        A1B = [cp.tile([128, 2, 16, 30], bf16, name=f"A1B{i}", tag=f"A1B{i}")
               for i in range(3)]
        A2B = [cp.tile([128, 2, 16, 30], bf16, name=f"A2B{i}", tag=f"A2B{i}")
               for i in range(3)]

        nc.sync.dma_start(W1T[:, :], w1_d.ap()[:, :])
        nc.gpsimd.dma_start(W2T[:, :, :], w2_d.ap()[:, :, :])
        nc.scalar.dma_start(W3T[:, :, :], w3_d.ap()[:, :, :])
        nc.sync.dma_start(B1T[:, :], b1_d.ap()[:, :])
        nc.gpsimd.dma_start(B2T[:, :], b2_d.ap()[:, :])
        nc.scalar.dma_start(B3T[:, :], b3_d.ap()[:, :])
        nc.gpsimd.dma_start(WFT[:, :], wf_d.ap()[:, :])
        nc.gpsimd.dma_start(WHT[:, :], wh_d.ap()[:, :])
        nc.scalar.dma_start(WOT[:, :], wo_d.ap()[:, :])
        nc.scalar.dma_start(BOT[:, :], bo_d.ap()[:, :])

        # Z rows 96:128 := 1.0 once (row 112 is the LSTM bias-ones row; rows
        # 0:112 are fully overwritten by pool accum_out before every read;
        # base partition must be a multiple of 32)
        for z in ZT:
            nc.vector.memset(z[96:128, :], 1.0)
        nc.vector.memset(CT[0][:, :], 0.0)
        nc.vector.memset(HT[0][:, :], 0.0)

        # one-time pad zeroing of the persistent act buffers:
        # half0 (p0:48): pad row 0; half1 (p64:112): pad row 15; cols 0, 29 both.
        for A in A1B + A2B:
            nc.gpsimd.memset(A[0:48, :, 0:1, :], 0.0)
            nc.gpsimd.memset(A[64:112, :, 15:16, :], 0.0)
            nc.gpsimd.memset(A[0:128, :, :, 0:1], 0.0)
            nc.gpsimd.memset(A[0:128, :, :, 29:30], 0.0)

        # ---- pools ----
        x1_pool = ctx.enter_context(tc.tile_pool(name="x1p", bufs=6))
        ps_pool = ctx.enter_context(tc.tile_pool(name="psp", bufs=6, space="PSUM"))
        g_pool = ctx.enter_context(tc.tile_pool(name="gp", bufs=2, space="PSUM"))
        ls_pool = ctx.enter_context(tc.tile_pool(name="lsp", bufs=4))

        # per-pair state carried between loop stages
        P1 = [None] * _PAIRS   # (PA, PB, mA, mB): bank = frame in pair
        P2 = [None] * _PAIRS
        P3 = [None] * _PAIRS

        def conv1(p):
            # block-diagonal K=18/M=112 packing: each streamed column carries
            # the 9 im2col taps of one frame; chunk rows 0..13 at lhsT rows
            # 0:9 -> psum 0:48, chunk rows 14..27 at rows 9:18 -> psum 64:112.
            # Bank PA = frame 0 of the pair, PB = frame 1.
            X = x1_pool.tile([128, 392], bf16, name="X")
            nc.sync.dma_start(X[0:18, :], x1_d.ap()[p, 0])
            nc.sync.dma_start(X[64:82, :], x1_d.ap()[p, 1])
            PAf = ps_pool.tile([128, 512], f32, name="PAf", tag="cps")
            PBf = ps_pool.tile([128, 512], f32, name="PBf", tag="cps")
            PA, PB = PAf[:, 0:392], PBf[:, 0:392]
            mA = nc.tensor.matmul(PA[0:112, :], lhsT=W1T[0:18, 0:112], rhs=X[0:18, :],
                                  skip_group_check=True)
            mB = nc.tensor.matmul(PB[0:112, :], lhsT=W1T[64:82, 0:112], rhs=X[64:82, :],
                                  skip_group_check=True)
            P1[p] = (PA, PB, mA, mB)

        def conv23(p, W, A, Pout):
            # bank = frame g. PA (frame 0): block 0:64 <- half0 (out rows
            # 0..13, lo), block 64:128 <- half1 (rows 14..27, hi). PB (frame
            # 1) is SWAPPED: block 0:64 <- half1 (rows 14..27, hi), block
            # 64:128 <- half0 (rows 0..13, lo) -- so the four matmuls of a
            # tap land on four distinct PE tiles (no same-tile back-to-back
            # streams, LDWEIGHTS stays in the background buffer).
            PAf = ps_pool.tile([128, 512], f32, name="PA23f", tag="cps")
            PBf = ps_pool.tile([128, 512], f32, name="PB23f", tag="cps")
            PA, PB = PAf[:, 0:392], PBf[:, 0:392]
            # dy=1 taps first: their windows (buf rows 1..14) only touch the
            # big relu_store writes, not the seam rows 0/15 -- so this pair's
            # first matmuls don't wait on the DVE seam chain.
            for ti, tap in enumerate((3, 4, 5, 0, 1, 2, 6, 7, 8)):
                dy, dx = tap // 3, tap % 3
                st = ti == 0
                sp = ti == 8
                lo = W[0:48, tap, :]
                hi = W[64:112, tap, :]
                # slot 1: tiles (0,0) + (64,64); slot 2: tiles (0,64) + (64,0)
                nc.tensor.matmul(PA[0:64, :], lhsT=lo,
                                 rhs=A[0:48, 0:1, dy:dy + 14, dx:dx + 28],
                                 start=st, stop=sp, skip_group_check=True)
                mA = nc.tensor.matmul(PA[64:128, :], lhsT=hi,
                                 rhs=A[64:112, 0:1, dy:dy + 14, dx:dx + 28],
                                 start=st, stop=sp, skip_group_check=True)
                mB = nc.tensor.matmul(PB[64:128, :], lhsT=lo,
                                 rhs=A[0:48, 1:2, dy:dy + 14, dx:dx + 28],
                                 start=st, stop=sp, skip_group_check=True)
                nc.tensor.matmul(PB[0:64, :], lhsT=hi,
                                 rhs=A[64:112, 1:2, dy:dy + 14, dx:dx + 28],
                                 start=st, stop=sp, skip_group_check=True)
            Pout[p] = (PA, PB, mA, mB)

        def relu_store(p, Psrc, B, Abufs, Adst_out):
            # psum (frame-banked) -> padded split act buffer, relu + bias.
            PA, PB, mA, mB = Psrc[p]
            A = Abufs[p % 3]
            # big write: whole frame, both halves in one [112]-partition op.
            # buf0 rows 1..14 <- out rows 0..13 (block 0:64); buf1 rows 1..14
            # <- out rows 14..27 (block 64:128); junk lands on partitions
            # 48:64 (unused gap).
            pa4 = PA.rearrange("p (a r c) -> p a r c", a=1, r=14, c=28)
            pb4 = PB.rearrange("p (a r c) -> p a r c", a=1, r=14, c=28)
            # frame 0 (PA, straight): one [112]-partition write covers both
            # halves (buf rows 1..14 in each); junk lands on partitions 48:64.
            i1 = nc.scalar.activation(A[0:112, 0:1, 1:15, 1:29], pa4[0:112, :, :, :],
                                      AF.Relu, bias=B[0:112, :])
            # frame 1 (PB, swapped blocks): two crossed half-writes. Both are
            # partition-base-shifted (in base != out base), which only the
            # DVE handles; the ACT takes the base-aligned seam slivers.
            i2a = nc.vector.tensor_scalar(A[0:48, 1:2, 1:15, 1:29],
                                          pb4[64:112, :, :, :],
                                          B[64:112, :], 0.0, ALU.add, ALU.max)
            i2b = nc.vector.tensor_scalar(A[64:112, 1:2, 1:15, 1:29],
                                          pb4[0:48, :, :, :],
                                          B[0:48, :], 0.0, ALU.add, ALU.max)
            # seam slivers: buf0 row 15 = out row 14 (first 28 cols of the
            # rows-14..27 block); buf1 row 0 = out row 13 (last 28 cols of
            # the rows-0..13 block).
            i3 = nc.vector.tensor_scalar(A[0:48, 0:1, 15:16, 1:29],
                                         pa4[64:112, :, 0:1, :],
                                         B[64:112, :], 0.0, ALU.add, ALU.max)
            i4 = nc.vector.tensor_scalar(A[64:112, 0:1, 0:1, 1:29],
                                         pa4[0:48, :, 13:14, :],
                                         B[0:48, :], 0.0, ALU.add, ALU.max)
            i5 = nc.scalar.activation(A[0:48, 1:2, 15:16, 1:29],
                                      pb4[0:48, :, 0:1, :],
                                      AF.Relu, bias=B[0:48, :])
            i6 = nc.scalar.activation(A[64:112, 1:2, 0:1, 1:29],
                                      pb4[64:112, :, 13:14, :],
                                      AF.Relu, bias=B[64:112, :])
            # PE-W + engine-R same-bank hazard: order every reader after the
            # tile's last matmul (PE completes in program order).
            for rd in (i1, i3, i4):
                add_dep_helper(rd.ins, mA.ins, reason="psum bank PA fully written")
            for rd in (i2a, i2b, i5, i6):
                add_dep_helper(rd.ins, mB.ins, reason="psum bank PB fully written")
            Adst_out[p] = A

        def pool3(p):
            # conv3 psum -> relu (in place) + per-frame accum straight into
            # the Z column for this (timestep, batch).
            PA, PB, mA, mB = P3[p]
            t, j = p // 2, p % 2
            Z = ZT[t % 3]
            ra = nc.scalar.activation(PA[0:112, :], PA[0:112, :],
                                      AF.Relu, bias=B3T[0:112, :],
                                      accum_out=Z[0:112, 2 * j:2 * j + 1])
            rb = nc.scalar.activation(PB[0:112, :], PB[0:112, :],
                                      AF.Relu, bias=B3T[0:112, :],
                                      accum_out=Z[0:112, 2 * j + 1:2 * j + 2])
            add_dep_helper(ra.ins, mA.ins, reason="psum bank PA fully written")
            add_dep_helper(rb.ins, mB.ins, reason="psum bank PB fully written")

        def lstm_step(t):
            # gate layout on psum partitions: f@0:8, i@32:40, o@64:72, g@96:104
            Z = ZT[t % 3]
            Hp, Hn = HT[t % 2], HT[(t + 1) % 2]
            Gf = g_pool.tile([128, 512], f32, name="Gf", tag="gps")
            G = Gf[:, 0:4]
            # K split skips Z rows 48:64 (junk accum from unused psum
            # partitions; could be non-finite at startup)
            nc.tensor.matmul(G[0:104, :], lhsT=WFT[0:48, 0:104], rhs=Z[0:48, :],
                             start=True, stop=False)
            nc.tensor.matmul(G[0:104, :], lhsT=WFT[64:113, 0:104], rhs=Z[64:113, :],
                             start=False, stop=False)
            nc.tensor.matmul(G[0:104, :], lhsT=WHT[0:8, 0:104], rhs=Hp[:, :],
                             start=False, stop=True)
            # each gate activation writes to a base-0 tile (tensor_tensor
            # requires all operands at the same start partition)
            GF = ls_pool.tile([8, 4], f32, name="GF")
            GI = ls_pool.tile([8, 4], f32, name="GI")
            GO = ls_pool.tile([8, 4], f32, name="GO")
            GG = ls_pool.tile([8, 4], f32, name="GG")
            nc.scalar.activation(GF[:, :], G[0:8, :], AF.Sigmoid)
            nc.scalar.activation(GI[:, :], G[32:40, :], AF.Sigmoid)
            nc.scalar.activation(GO[:, :], G[64:72, :], AF.Sigmoid)
            nc.scalar.activation(GG[:, :], G[96:104, :], AF.Tanh)
            Cp, Cn = CT[t % 2], CT[(t + 1) % 2]
            T1 = ls_pool.tile([8, 4], f32, name="T1")
            T2 = ls_pool.tile([8, 4], f32, name="T2")
            nc.vector.tensor_mul(T1[:, :], GF[:, :], Cp[:, :])
            nc.vector.tensor_mul(T2[:, :], GI[:, :], GG[:, :])
            nc.vector.tensor_add(Cn[:, :], T1[:, :], T2[:, :])
            TC = ls_pool.tile([8, 4], f32, name="TC")
            nc.scalar.activation(TC[:, :], Cn[:, :], AF.Tanh)
            nc.vector.tensor_mul(Hn[:, :], GO[:, :], TC[:, :])
            nc.vector.tensor_mul(HST[0:8, 4 * t:4 * t + 4], GO[:, :], TC[:, :])

        # ---- software-pipelined emission ----
        A1 = [None] * _PAIRS
        A2 = [None] * _PAIRS
        for p in range(_PAIRS + 3):
            # pool3 first: its ACTs free the psum banks this step's conv3
            # will reuse (6-buffer rotation), so queue them ahead of the
            # relu stores on the Scalar engine.
            if 3 <= p < _PAIRS + 3:
                pool3(p - 3)
            if p < _PAIRS:
                conv1(p)
            if 1 <= p < _PAIRS + 1:
                q = p - 1
                relu_store(q, P1, B1T, A1B, A1)
                conv23(q, W2T, A1[q], P2)
            if 2 <= p < _PAIRS + 2:
                q = p - 2
                relu_store(q, P2, B2T, A2B, A2)
                conv23(q, W3T, A2[q], P3)
            if p >= 4 and p % 2 == 0:
                lstm_step((p - 4) // 2)

        # ---- output head ----
        Yf = g_pool.tile([128, 512], f32, name="Yf", tag="gps")
        Y = Yf[0:1, 0:128]
        nc.tensor.matmul(Y[:, :], lhsT=WOT[0:8, :], rhs=HST[0:8, :])
        OUTS = cp.tile([1, 128], f32, name="OUTS", tag="OUTS")
        nc.vector.tensor_scalar(OUTS[:, :], Y[:, :], BOT[0:1, :], None, ALU.add)
        nc.sync.dma_start(out_d.ap()[:, :], OUTS[:, :])

    nc.compile()
    return nc


# ---------------------------------------------------------------------------
# Host-side prep
# ---------------------------------------------------------------------------

def _prep_core_inputs(xc, w1, b1, w2, b2, w3, b3, bn1, wf, bf, wi1, bi1, wi2, bi2,
                      wo, bo, bn2, w_out, b_out):
    """xc: [4, 32, 28, 28, 1] float32 for one core. Returns the in_map dict."""
    T, HW = _T, _HW
    xp = np.zeros((_BPC, T, 30, 30), _F32)
    xp[:, :, 1:29, 1:29] = xc[..., 0]

    # im2col for conv1, frame-major block-diagonal: for frame g of pair
    # (t, j) (batch b = 2j+g), rows 0:9 = taps of out rows 0..13, rows 9:18 =
    # taps of out rows 14..27.
    X1 = np.empty((_PAIRS, 2, 18, 392), _F32)
    X1v = X1.reshape(T, 2, 2, 18, 14, 28)  # [t, j, g, krow, r, c]
    for dy in range(3):
        for dx in range(3):
            blkA = xp[:, :, dy:dy + 14, dx:dx + 28]        # [b, t, 14, 28]
            blkB = xp[:, :, 14 + dy:28 + dy, dx:dx + 28]
            for j in range(2):
                for g in range(2):
                    X1v[:, j, g, 3 * dy + dx] = blkA[2 * j + g]
                    X1v[:, j, g, 9 + 3 * dy + dx] = blkB[2 * j + g]

    def wpack1():
        # block diagonal. Frame 0 (K rows 0:18): chunk A (out rows 0..13) ->
        # out cols 0:48, chunk B -> cols 64:112. Frame 1 (K rows 64:82) is
        # SWAPPED to match the conv23 PB tile convention: chunk A -> cols
        # 64:112, chunk B -> cols 0:48.
        w = np.zeros((128, 128), _F32)
        w9 = w1.reshape(9, _F)
        w[0:9, 0:48] = w9
        w[9:18, 64:112] = w9
        w[64:73, 64:112] = w9
        w[73:82, 0:48] = w9
        return w

    def wpack(wn):
        w = np.zeros((128, 9, 64), _F32)
        for tap in range(9):
            m = wn[tap // 3, tap % 3]  # [48, 48]
            w[0:48, tap, 0:48] = m
            w[64:112, tap, 0:48] = m
        return w

    def bpack(bn):
        b = np.zeros((128, 1), _F32)
        b[0:48, 0] = bn
        b[64:112, 0] = bn
        return b

    bn1_g, bn1_b, bn1_m, bn1_v = bn1
    bn2_g, bn2_b, bn2_m, bn2_v = bn2
    s1 = bn1_g / np.sqrt(bn1_v + _EPS)
    t1 = bn1_b - bn1_m * s1
    Wx = np.concatenate([wf[:_F], wi1[:_F], wo[:_F], wi2[:_F]], axis=1)  # [48, 32] f,i,o,g
    Wh = np.concatenate([wf[_F:], wi1[_F:], wo[_F:], wi2[_F:]], axis=1)  # [8, 32]
    bias = np.concatenate([bf, bi1, bo, bi2]) + t1 @ Wx                  # [32]
    Wxs = (s1[:, None] * Wx) / float(HW * HW)

    def spread(m):
        # [r, 32] gate-packed (f,i,o,g x 8) -> [r, 128] at cols f@0:8, i@32:40,
        # o@64:72, g@96:104
        out = np.zeros((m.shape[0], 128), _F32)
        for gidx in range(4):
            out[:, 32 * gidx:32 * gidx + 8] = m[:, 8 * gidx:8 * gidx + 8]
        return out

    WF = np.zeros((128, 128), _F32)
    WF[0:48] = spread(Wxs)
    WF[64:112] = spread(Wxs)
    WF[112] = spread(bias[None, :])[0]     # Z row 112 is the ones row
    WH = np.zeros((128, 128), _F32)
    WH[0:8] = spread(Wh)

    s2 = bn2_g / np.sqrt(bn2_v + _EPS)
    t2 = bn2_b - bn2_m * s2
    WO = np.zeros((128, 1), _F32)
    WO[0:8, 0] = s2 * w_out[:, 0]
    bot = np.array([[t2 @ w_out[:, 0] + b_out[0]]], _F32)

    return {
        "x1": X1.astype(_BF16),
        "w1t": wpack1().astype(_BF16),
        "w2t": wpack(w2).astype(_BF16), "w3t": wpack(w3).astype(_BF16),
        "b1t": bpack(b1), "b2t": bpack(b2), "b3t": bpack(b3),
        "wft": WF, "wht": WH, "woutt": WO, "bot": bot,
    }


_PROG = None
_LAST_RESULTS = None


def _install_ntff_hook():
    """The agent image's antenv lacks axon_hooks; synthesize it and register
    the ctypes-based NTFF profile hook from trn_agent_boot."""
    import types
    import antenv
    if getattr(antenv, "axon_hooks", None) is not None:
        return
    m = types.ModuleType("antenv.axon_hooks")
    state = {"h": None}
    m.set_axon_ntff_profile_hook = lambda h: state.__setitem__("h", h)
    m.get_axon_ntff_profile_hook = lambda: state["h"]
    sys.modules["antenv.axon_hooks"] = m
    antenv.axon_hooks = m
    try:
        from trn_agent_boot.trn_boot import _ntff_profile_via_ctypes
        m.set_axon_ntff_profile_hook(_ntff_profile_via_ctypes("/opt/axon/libaxon_pjrt.so"))
    except Exception as e:
        print("ntff hook install failed:", e)


def kernel(**inputs):
    global _PROG
    inp = {k: np.asarray(v, dtype=np.asarray(v).dtype) for k, v in inputs.items()}
    x = np.asarray(inp["x"], _F32)
    w2 = np.asarray(inp["w2"], _F32)
    w3 = np.asarray(inp["w3"], _F32)
    bn1 = tuple(np.asarray(inp[k], _F32) for k in ("bn1_g", "bn1_b", "bn1_m", "bn1_v"))
    bn2 = tuple(np.asarray(inp[k], _F32) for k in ("bn2_g", "bn2_b", "bn2_m", "bn2_v"))

    in_maps = []
    for c in range(_NCORES):
        xc = x[c * _BPC:(c + 1) * _BPC]
        in_maps.append(_prep_core_inputs(
            xc, np.asarray(inp["w1"], _F32), np.asarray(inp["b1"], _F32),
            w2, np.asarray(inp["b2"], _F32), w3, np.asarray(inp["b3"], _F32),
            bn1,
            np.asarray(inp["wf"], _F32), np.asarray(inp["bf"], _F32),
            np.asarray(inp["wi1"], _F32), np.asarray(inp["bi1"], _F32),
            np.asarray(inp["wi2"], _F32), np.asarray(inp["bi2"], _F32),
            np.asarray(inp["wo"], _F32), np.asarray(inp["bo"], _F32),
            bn2, np.asarray(inp["w_out"], _F32), np.asarray(inp["b_out"], _F32),
        ))

    if _PROG is None:
        _PROG = _build_program()
    from concourse.bass_utils import run_bass_kernel_spmd
    import os as _os
    if _os.environ.get("TRN_KERNEL_TRACE"):
        _install_ntff_hook()
    res = run_bass_kernel_spmd(_PROG, in_maps, core_ids=list(range(_NCORES)),
                               trace=bool(_os.environ.get("TRN_KERNEL_TRACE")))
    global _LAST_RESULTS
    _LAST_RESULTS = res

    out = np.empty((_B, _T, 1), _F32)
    for c in range(_NCORES):
        yc = res.results[c]["out"].reshape(_T, _BPC).T  # [4, 32]
        out[c * _BPC:(c + 1) * _BPC, :, 0] = yc
    return out


if __name__ == "__main__":
    pass
